# revision 1
# baseline (speedup 1.0000x reference)
"""Multi-head attention (B=2, S=2048, D=1024, H=16) on 8 Trainium2 cores.

Sharding: core c -> (batch b = c//4, head-group hg = c%4 of 4 heads, d_h=256).
Megatron-style: column-shard W_{q,k,v}, row-shard W_o; partial outputs are
summed on the host (the unshard step).

Per-core pipeline (activations kept transposed, "T-space", fp32r matmuls):
  kT = (Wk_hg/8) @ K_b^T        [256, 2048]
  v  = V_b @ Wv_hg^T (+ones col)[2048, 4*65]
  per 512-wide q-chunk qc:
    qT[:, qc] = Wq_hg @ Q_b^T[:, qc]
    per head pair, per kt pair: sT[kpos, q] = kT_h-slices^T @ qT_h (row-packed)
    pT = exp(sT)            (ScalarE, 1024-wide over 2 PSUM banks)
    ctxT (+denom row) = v_h_aug^T @ pT   (M=65, accumulated over 16 kt)
    ctxT /= denom           (DVE recip + DRAM-roundtrip partition broadcast)
    oT_partial[:, qc] = Wo_cols^T-slices @ ctxT
Host: out[b] = (sum over the 4 cores of batch b of oT).T + bo.
"""

import numpy as np

import bass_rust
import concourse.bass as bass
import concourse.mybir as mybir
import concourse.tile as tile
from concourse.bass_utils import run_bass_kernel_spmd

F32 = mybir.dt.float32
F32R = mybir.dt.float32r

B, S, D = 2, 2048, 1024
H = 16
DK = 64
N_CORES = 8
HEADS_PER_CORE = 4          # d_h = 256
DH = HEADS_PER_CORE * DK    # 256
VW = DK + 1                 # v columns per head incl. ones column
NV = HEADS_PER_CORE * VW    # 260
QC = 512                    # q-chunk (PSUM bank = 512 fp32)
N_QC = S // QC              # 4
N_KT = S // 128             # 16 key tiles
N_KO = D // 128             # 8 contraction tiles for projections
MT = DH // 128              # 2 m-tiles for qT/kT/ctxT


def _legalize_waits(nc):
    """walrus here allows 1 sync-wait per instruction (2 for EventSemaphore);
    Tile emits more. Spill extras onto same-engine NoOps placed just before."""
    caps = {"InstEventSemaphore": 2}
    n_nops = 0
    for f in nc.m.functions:
        for bb in f.blocks:
            insts = bb.instructions
            out = []
            changed = False
            for inst in insts:
                si = inst.sync_info
                waits = list(si.on_wait) if si is not None else []
                cap = caps.get(type(inst).__name__, 1)
                if len(waits) > cap:
                    spill, keep = waits[:-cap], waits[-cap:]
                    for w in spill:
                        nop = mybir.InstNoOp(name=f"waitfix-{n_nops}", ins=[], outs=[])
                        n_nops += 1
                        nop.engine = inst.engine
                        nop.sync_info = bass_rust.SyncInfo(on_wait=[w], on_update=[])
                        out.append(nop)
                    si.on_wait = keep
                    changed = True
                out.append(inst)
            if changed:
                insts[:] = out
    return n_nops


def build_nc(reps: int = 1):
    nc = bass.Bass(num_devices=N_CORES)

    t = {}
    t["xqT"] = nc.dram_tensor("xqT", [D, S], F32, kind="ExternalInput")
    t["xkT"] = nc.dram_tensor("xkT", [D, S], F32, kind="ExternalInput")
    t["xvT"] = nc.dram_tensor("xvT", [D, S], F32, kind="ExternalInput")
    t["wqT"] = nc.dram_tensor("wqT", [D, DH], F32, kind="ExternalInput")
    t["wkT"] = nc.dram_tensor("wkT", [D, DH], F32, kind="ExternalInput")
    t["wvT"] = nc.dram_tensor("wvT", [D, NV], F32, kind="ExternalInput")
    t["bq"] = nc.dram_tensor("bq", [DH], F32, kind="ExternalInput")
    t["bk"] = nc.dram_tensor("bk", [DH], F32, kind="ExternalInput")
    t["bv_bc"] = nc.dram_tensor("bv_bc", [128, NV], F32, kind="ExternalInput")
    t["woT"] = nc.dram_tensor("woT", [DH, D], F32, kind="ExternalInput")
    t["oT"] = nc.dram_tensor("oT", [D, S], F32, kind="ExternalOutput")

    with tile.TileContext(nc) as tc:
        _body(nc, tc, t, reps)
    _legalize_waits(nc)
    return nc


def _body(nc, tc, t, reps):
    from contextlib import ExitStack

    with ExitStack() as ctx:
        singles = ctx.enter_context(tc.tile_pool(name="singles", bufs=1))

        wq_s = singles.tile([128, N_KO, DH], F32R)
        wk_s = singles.tile([128, N_KO, DH], F32R)
        wv_s = singles.tile([128, N_KO, NV], F32R)
        wo_s = singles.tile([128, MT, D], F32R)
        bq_s = singles.tile([128, MT], F32)
        bk_s = singles.tile([128, MT], F32)
        bv_s = singles.tile([128, NV], F32)
        nc.sync.dma_start(wq_s[:], t["wqT"].rearrange("(ko p) m -> p ko m", p=128).bitcast(F32R))
        nc.sync.dma_start(wk_s[:], t["wkT"].rearrange("(ko p) m -> p ko m", p=128).bitcast(F32R))
        nc.sync.dma_start(wv_s[:], t["wvT"].rearrange("(ko p) m -> p ko m", p=128).bitcast(F32R))
        nc.sync.dma_start(wo_s[:], t["woT"].rearrange("(kt p) e -> p kt e", p=128).bitcast(F32R))
        nc.sync.dma_start(bq_s[:], t["bq"].rearrange("(m p) -> p m", p=128))
        nc.sync.dma_start(bk_s[:], t["bk"].rearrange("(m p) -> p m", p=128))
        nc.sync.dma_start(bv_s[:], t["bv_bc"][:, :])

        qT_s = singles.tile([128, MT, S], F32R)
        kT_s = singles.tile([128, MT, S], F32R)
        v_s = singles.tile([128, N_KT, NV], F32R)
        ctxT_s = singles.tile([128, MT, S], F32R)

        for _ in range(reps):
            with ExitStack() as ictx:
                _compute(nc, tc, ictx, t, wq_s, wk_s, wv_s, bq_s, bk_s, bv_s,
                         wo_s, qT_s, kT_s, v_s, ctxT_s)


def _proj_qk(nc, xin, proj_ps, xdram, w_s, b_s, dst, sc):
    """Project one 512-col chunk of qT or kT: dst[:, :, sc*QC:+QC]."""
    xt = xin.tile([128, N_KO, QC], F32R, tag="x", name="xt")
    nc.sync.dma_start(
        xt[:],
        xdram.rearrange("(ko p) s -> p ko s", p=128)[
            :, :, sc * QC:(sc + 1) * QC
        ].bitcast(F32R),
    )
    for m in range(MT):
        ps = proj_ps.tile([128, QC], F32, tag="proj", name="proj_ps")
        for ko in range(N_KO):
            nc.tensor.matmul(
                ps[:],
                w_s[:, ko, m * 128:(m + 1) * 128],
                xt[:, ko],
                start=(ko == 0),
                stop=(ko == N_KO - 1),
            )
        nc.vector.tensor_tensor(
            dst[:, m, sc * QC:(sc + 1) * QC],
            ps[:],
            b_s[:, m, None].to_broadcast((128, QC)),
            mybir.AluOpType.add,
        )


def _proj_v(nc, xin, proj_ps, xvT, wv_s, bv_s, v_s, sc):
    xt = xin.tile([128, N_KO, QC], F32R, tag="x", name="xt")
    nc.sync.dma_start(
        xt[:],
        xvT.rearrange("(ko p) s -> p ko s", p=128)[
            :, :, sc * QC:(sc + 1) * QC
        ].bitcast(F32R),
    )
    for rt in range(QC // 128):
        ps = proj_ps.tile([128, QC], F32, tag="proj", name="proj_ps")
        for ko in range(N_KO):
            nc.tensor.matmul(
                ps[:, :NV],
                xt[:, ko, rt * 128:(rt + 1) * 128],
                wv_s[:, ko],
                start=(ko == 0),
                stop=(ko == N_KO - 1),
            )
        nc.vector.tensor_tensor(
            v_s[:, sc * (QC // 128) + rt, :],
            ps[:, :NV],
            bv_s[:],
            mybir.AluOpType.add,
        )


def _compute(nc, tc, ctx, t, wq_s, wk_s, wv_s, bq_s, bk_s, bv_s, wo_s,
             qT_s, kT_s, v_s, ctxT_s):
    # ---------- Stage A: projections ----------
    with tc.tile_pool(name="xin", bufs=3) as xin, \
         tc.tile_pool(name="proj_ps", bufs=4, space="PSUM") as proj_ps:
        for sc in range(N_QC):
            _proj_qk(nc, xin, proj_ps, t["xqT"], wq_s, bq_s, qT_s, sc)
        for sc in range(N_QC):
            _proj_qk(nc, xin, proj_ps, t["xkT"], wk_s, bk_s, kT_s, sc)
        for sc in range(N_QC):
            _proj_v(nc, xin, proj_ps, t["xvT"], wv_s, bv_s, v_s, sc)

    # ---------- Stage B: attention + output projection ----------
    with tc.tile_pool(name="pT", bufs=3) as pT_pool, \
         tc.tile_pool(name="norm", bufs=4) as norm_pool, \
         tc.tile_pool(name="ndram", bufs=4, space="DRAM") as ndram_pool, \
         tc.tile_pool(name="osb", bufs=2) as osb_pool, \
         tc.tile_pool(name="sc_ps", bufs=2, space="PSUM") as sc_ps, \
         tc.tile_pool(name="ctx_ps", bufs=1, space="PSUM") as ctx_ps, \
         tc.tile_pool(name="o_ps", bufs=2, space="PSUM") as o_ps:
        for qc in range(N_QC):
            for pair in range(HEADS_PER_CORE // 2):
                m = pair
                ctx_banks = [
                    ctx_ps.tile([128, QC], F32, tag=f"ctx{hl}", name=f"ctx{hl}")
                    for hl in range(2)
                ]
                # software pipeline: scores+exp for kt, PV consumes kt-1's
                # exp output so PE never waits on the ScalarE exp latency.
                pend = [None, None]  # per hl: (pt_tile, kt)
                for kt in range(N_KT):
                    for hl in range(2):
                        off = 64 * hl
                        ps_s = sc_ps.tile([128, QC], F32, tag=f"s{hl}", name=f"s{hl}")
                        nc.tensor.matmul(
                            ps_s[:],
                            kT_s[off:off + 64, m, kt * 128:(kt + 1) * 128],
                            qT_s[off:off + 64, m, qc * QC:(qc + 1) * QC],
                            start=True,
                            stop=True,
                        )
                        pt = pT_pool.tile([128, QC], F32R, tag=f"p{hl}", name=f"pt{hl}")
                        nc.scalar.activation(
                            pt[:], ps_s[:], mybir.ActivationFunctionType.Exp
                        )
                        prev = pend[hl]
                        pend[hl] = (pt, kt)
                        if prev is not None:
                            h = 2 * pair + hl
                            nc.tensor.matmul(
                                ctx_banks[hl][0:VW, :],
                                v_s[:, prev[1], h * VW:(h + 1) * VW],
                                prev[0][:],
                                start=(prev[1] == 0),
                                stop=False,
                            )
                for hl in range(2):
                    h = 2 * pair + hl
                    pt, kt = pend[hl]
                    nc.tensor.matmul(
                        ctx_banks[hl][0:VW, :],
                        v_s[:, kt, h * VW:(h + 1) * VW],
                        pt[:],
                        start=False,
                        stop=True,
                    )
                for hl in range(2):
                    r_s = norm_pool.tile([1, QC], F32, tag="r", name="r_s")
                    nc.vector.reciprocal(r_s[:], ctx_banks[hl][64:65, :])
                    r_d = ndram_pool.tile([1, QC], F32, tag="rd", name="r_d")
                    nc.gpsimd.dma_start(r_d[:], r_s[:])
                    rbc = norm_pool.tile([64, QC], F32, tag="rbc", name="rbc")
                    nc.gpsimd.dma_start(
                        rbc[:],
                        bass.AP(
                            tensor=r_d.tensor,
                            offset=r_d.offset,
                            ap=[[0, 64]] + list(r_d.ap[1:]),
                        ),
                    )
                    nc.vector.tensor_tensor(
                        ctxT_s[64 * hl:64 * hl + 64, m, qc * QC:(qc + 1) * QC],
                        ctx_banks[hl][0:64, :],
                        rbc[:],
                        mybir.AluOpType.mult,
                    )
            o_sb = osb_pool.tile([128, D // 128, QC], F32, tag="o", name="o_sb")
            for mt in range(D // 128):
                ps_o = o_ps.tile([128, QC], F32, tag="o", name="ps_o")
                for kt in range(MT):
                    nc.tensor.matmul(
                        ps_o[:],
                        wo_s[:, kt, mt * 128:(mt + 1) * 128],
                        ctxT_s[:, kt, qc * QC:(qc + 1) * QC],
                        start=(kt == 0),
                        stop=(kt == MT - 1),
                    )
                nc.vector.tensor_copy(o_sb[:, mt, :], ps_o[:])
            nc.sync.dma_start(
                t["oT"].rearrange("(mt p) s -> p mt s", p=128)[
                    :, :, qc * QC:(qc + 1) * QC
                ],
                o_sb[:],
            )


def shard_inputs(Q, K, V, Wq, bq, Wk, bk, Wv, bv, Wo, bo):
    """Host-side shard prep. Returns per-core in_maps."""
    scale = 1.0 / np.sqrt(np.float32(DK))
    in_maps = []
    xT = {}
    for b in range(B):
        xT[b] = (
            np.ascontiguousarray(np.asarray(Q[b]).T),
            np.ascontiguousarray(np.asarray(K[b]).T),
            np.ascontiguousarray(np.asarray(V[b]).T),
        )
    for c in range(N_CORES):
        b, hg = c // HEADS_PER_CORE, c % HEADS_PER_CORE
        rows = slice(DH * hg, DH * (hg + 1))
        wqT = np.ascontiguousarray(np.asarray(Wq)[rows].T)
        wkT = np.ascontiguousarray((np.asarray(Wk)[rows] * scale).T)
        wvT = np.zeros((D, NV), np.float32)
        bv_bc = np.zeros((128, NV), np.float32)
        for i in range(HEADS_PER_CORE):
            wr = slice(DH * hg + DK * i, DH * hg + DK * (i + 1))
            wvT[:, VW * i:VW * i + DK] = np.asarray(Wv)[wr].T
            bv_bc[:, VW * i:VW * i + DK] = np.asarray(bv)[wr][None, :]
            bv_bc[:, VW * i + DK] = 1.0
        woT = np.ascontiguousarray(np.asarray(Wo)[:, rows].T)
        in_maps.append(
            {
                "xqT": xT[b][0],
                "xkT": xT[b][1],
                "xvT": xT[b][2],
                "wqT": wqT,
                "wkT": wkT,
                "wvT": wvT,
                "bq": np.ascontiguousarray(np.asarray(bq)[rows]),
                "bk": np.ascontiguousarray(np.asarray(bk)[rows] * scale),
                "bv_bc": bv_bc,
                "woT": woT,
            }
        )
    return in_maps


def unshard(results, bo):
    out = np.empty((B, S, D), np.float32)
    for b in range(B):
        acc = results[b * HEADS_PER_CORE]["oT"].astype(np.float32).copy()
        for hg in range(1, HEADS_PER_CORE):
            acc += results[b * HEADS_PER_CORE + hg]["oT"]
        out[b] = acc.T + np.asarray(bo)[None, :]
    return out


_NC_CACHE = {}


def kernel(Q, K, V, Wq, bq, Wk, bk, Wv, bv, Wo, bo):
    if "nc" not in _NC_CACHE:
        _NC_CACHE["nc"] = build_nc()
    nc = _NC_CACHE["nc"]
    in_maps = shard_inputs(Q, K, V, Wq, bq, Wk, bk, Wv, bv, Wo, bo)
    res = run_bass_kernel_spmd(nc, in_maps, core_ids=list(range(N_CORES)))
    return unshard(res.results, bo)



# revision 18
# speedup vs baseline: 1.3408x; 1.3408x over previous
"""Multi-head attention (B=2, S=2048, D=1024, H=16) on 8 Trainium2 cores.

Sharding: core c -> (batch b = c//4, head-group hg = c%4 of 4 heads, d_h=256).
Megatron-style: column-shard W_{q,k,v}, row-shard W_o; partial outputs are
summed on the host (the unshard step).

v2 restructure vs baseline: the attention inner loop is ACT(exp)-bound
(~1.04us/kt for a head-pair vs ~0.85us of PE work), so the kernel is
organized to keep ScalarE saturated with 2-bank exps ([128,1024] over both
heads of a pair) while PE fills its slack with "filler" matmuls: projections
of later chunks and the previous qc's output projection, interleaved into
the score/PV stream.  Stage-A evacuations (bias adds) run on GPSIMD (Pool),
input DMAs are spread over the ACT/DVE/Pool/SP queues in chase order.

Per-core pipeline (activations transposed "T-space", fp32r matmuls):
  kT = (Wk_hg/8) @ K_b^T        [256, 2048]
  v  = V_b @ Wv_hg^T (+ones col)[2048, 4*65]
  per 512-wide q-chunk qc, per head pair:
    per kt (128 keys): sT[2 heads] -> one 2-bank PSUM tile; exp -> pT;
    PV: ctx_h += v_h^T @ pT_h (M=65 incl denom row)
    ctx_h /= denom (DVE recip + DRAM-roundtrip partition broadcast)
  oT_partial[:, qc] = Wo_cols^T @ ctxT
Host: out[b] = (sum over the 4 cores of batch b of oT).T + bo.
"""

import numpy as np

import bass_rust
import concourse.bass as bass
import concourse.mybir as mybir
import concourse.tile as tile
from concourse.bass_utils import run_bass_kernel_spmd

F32 = mybir.dt.float32
F32R = mybir.dt.float32r

B, S, D = 2, 2048, 1024
H = 16
DK = 64
N_CORES = 8
HEADS_PER_CORE = 4          # d_h = 256
DH = HEADS_PER_CORE * DK    # 256
VW = DK + 1                 # v columns per head incl. ones column
NV = HEADS_PER_CORE * VW    # 260
QC = 512                    # q-chunk (PSUM bank = 512 fp32)
N_QC = S // QC              # 4
N_KT = S // 128             # 16 key tiles
N_KO = D // 128             # 8 contraction tiles for projections
MT = DH // 128              # 2 m-tiles
HC = 256                    # x half-chunk width (s columns)
N_HC = S // HC              # 8 half-chunks per input tensor


def _legalize_waits(nc):
    """walrus here allows 1 sync-wait per instruction (2 for EventSemaphore);
    Tile emits more. Spill extras onto same-engine NoOps placed just before."""
    caps = {"InstEventSemaphore": 2}
    n_nops = 0
    for f in nc.m.functions:
        for bb in f.blocks:
            insts = bb.instructions
            out = []
            changed = False
            for inst in insts:
                si = inst.sync_info
                waits = list(si.on_wait) if si is not None else []
                cap = caps.get(type(inst).__name__, 1)
                if len(waits) > cap:
                    spill, keep = waits[:-cap], waits[-cap:]
                    for w in spill:
                        nop = mybir.InstNoOp(name=f"waitfix-{n_nops}", ins=[], outs=[])
                        n_nops += 1
                        nop.engine = inst.engine
                        nop.sync_info = bass_rust.SyncInfo(on_wait=[w], on_update=[])
                        out.append(nop)
                    si.on_wait = keep
                    changed = True
                out.append(inst)
            if changed:
                insts[:] = out
    return n_nops


def build_nc(reps: int = 1):
    nc = bass.Bass(num_devices=N_CORES)

    t = {}
    t["xqT"] = nc.dram_tensor("xqT", [D, S], F32, kind="ExternalInput")
    t["xkT"] = nc.dram_tensor("xkT", [D, S], F32, kind="ExternalInput")
    t["xvT"] = nc.dram_tensor("xvT", [D, S], F32, kind="ExternalInput")
    t["wqT"] = nc.dram_tensor("wqT", [D, DH], F32, kind="ExternalInput")
    t["wkT"] = nc.dram_tensor("wkT", [D, DH], F32, kind="ExternalInput")
    t["wvT"] = nc.dram_tensor("wvT", [D, NV], F32, kind="ExternalInput")
    t["bq"] = nc.dram_tensor("bq", [DH], F32, kind="ExternalInput")
    t["bk"] = nc.dram_tensor("bk", [DH], F32, kind="ExternalInput")
    t["bv_bc"] = nc.dram_tensor("bv_bc", [128, NV], F32, kind="ExternalInput")
    t["woT"] = nc.dram_tensor("woT", [DH, D], F32, kind="ExternalInput")
    t["oT"] = nc.dram_tensor("oT", [D, S], F32, kind="ExternalOutput")

    with tile.TileContext(nc) as tc:
        _body(nc, tc, t, reps)
    _legalize_waits(nc)
    return nc


def _body(nc, tc, t, reps):
    from contextlib import ExitStack

    with ExitStack() as ctx:
        singles = ctx.enter_context(tc.tile_pool(name="singles", bufs=1))

        g = {}
        g["wq"] = singles.tile([128, N_KO, DH], F32R, name="wq_s")
        g["wk"] = singles.tile([128, N_KO, DH], F32R, name="wk_s")
        g["wv"] = singles.tile([128, N_KO, NV], F32R, name="wv_s")
        g["wo"] = singles.tile([128, MT, D], F32R, name="wo_s")
        g["bq"] = singles.tile([128, MT], F32, name="bq_s")
        g["bk"] = singles.tile([128, MT], F32, name="bk_s")
        g["bv"] = singles.tile([128, NV], F32, name="bv_s")
        # weights/biases: SP queue, ordered by first use
        nc.sync.dma_start(g["bq"][:], t["bq"].rearrange("(m p) -> p m", p=128))
        nc.sync.dma_start(g["bk"][:], t["bk"].rearrange("(m p) -> p m", p=128))
        nc.sync.dma_start(g["bv"][:], t["bv_bc"][:, :])
        nc.sync.dma_start(g["wk"][:], t["wkT"].rearrange("(ko p) m -> p ko m", p=128).bitcast(F32R))
        nc.sync.dma_start(g["wv"][:], t["wvT"].rearrange("(ko p) m -> p ko m", p=128).bitcast(F32R))

        g["kT"] = singles.tile([128, MT, S], F32R, name="kT_s")
        g["v"] = singles.tile([128, N_KT, NV], F32R, name="v_s")

        for _ in range(reps):
            with ExitStack() as ictx:
                _compute(nc, tc, ictx, t, g)


def _x_chunk(xdram, c):
    return xdram.rearrange("(ko p) s -> p ko s", p=128)[
        :, :, c * HC:(c + 1) * HC
    ].bitcast(F32R)


def _compute(nc, tc, ctx, t, g):
    wq_s, wk_s, wv_s, wo_s = g["wq"], g["wk"], g["wv"], g["wo"]
    bq_s, bk_s, bv_s = g["bq"], g["bk"], g["bv"]
    kT_s, v_s = g["kT"], g["v"]

    xk_p = ctx.enter_context(tc.tile_pool(name="xk", bufs=3))
    xv_p = ctx.enter_context(tc.tile_pool(name="xv", bufs=3))
    xq_p = ctx.enter_context(tc.tile_pool(name="xq", bufs=2))
    qt_p = ctx.enter_context(tc.tile_pool(name="qt", bufs=2))
    pt_p = ctx.enter_context(tc.tile_pool(name="pt", bufs=6))
    cxt_p = ctx.enter_context(tc.tile_pool(name="cxt", bufs=2))
    osb_p = ctx.enter_context(tc.tile_pool(name="osb", bufs=1))
    norm_p = ctx.enter_context(tc.tile_pool(name="norm", bufs=4))
    ndram_p = ctx.enter_context(tc.tile_pool(name="ndram", bufs=4, space="DRAM"))
    sc_ps = ctx.enter_context(tc.tile_pool(name="sc_ps", bufs=2, space="PSUM"))
    ctx_ps = ctx.enter_context(tc.tile_pool(name="ctx_ps", bufs=1, space="PSUM"))
    aux_ps = ctx.enter_context(tc.tile_pool(name="aux_ps", bufs=2, space="PSUM"))

    # ---------------- input DMAs (queue order = chase order) ----------------
    xk_t = [xk_p.tile([128, N_KO, HC], F32R, tag="xk", name=f"xk{c}")
            for c in range(N_HC)]
    xv_t = [xv_p.tile([128, N_KO, HC], F32R, tag="xv", name=f"xv{c}")
            for c in range(N_HC)]
    xq_t = [xq_p.tile([128, N_KO, HC], F32R, tag="xq", name=f"xq{c}")
            for c in range(N_HC)]
    for c in range(5):                       # xk c0-4 on ACT queue
        nc.scalar.dma_start(xk_t[c][:], _x_chunk(t["xkT"], c))
    for c in range(2):                       # xv c0-1 on Pool
        nc.gpsimd.dma_start(xv_t[c][:], _x_chunk(t["xvT"], c))
    nc.gpsimd.dma_start(wq_s[:], t["wqT"].rearrange("(ko p) m -> p ko m", p=128).bitcast(F32R))
    for c in range(2, 6):                    # xv c2-5 on Pool
        nc.gpsimd.dma_start(xv_t[c][:], _x_chunk(t["xvT"], c))
    # SP: xq qc0 first (attention gate), xk c5-7, then the rest by deadline
    for c in range(2):
        nc.sync.dma_start(xq_t[c][:], _x_chunk(t["xqT"], c))
    for c in range(5, N_HC):
        nc.sync.dma_start(xk_t[c][:], _x_chunk(t["xkT"], c))
    for c in range(2, 4):
        nc.sync.dma_start(xq_t[c][:], _x_chunk(t["xqT"], c))
    for c in range(2):
        nc.sync.dma_start(xv_t[6 + c][:], _x_chunk(t["xvT"], 6 + c))
    for c in range(4, 6):
        nc.sync.dma_start(xq_t[c][:], _x_chunk(t["xqT"], c))
    nc.sync.dma_start(wo_s[:], t["woT"].rearrange("(kt p) e -> p kt e", p=128).bitcast(F32R))
    for c in range(6, N_HC):
        nc.sync.dma_start(xq_t[c][:], _x_chunk(t["xqT"], c))

    # ------------- stage-A helpers -------------
    def proj_qk(c, x_t, w_s, b_s, dst, dst_c, evac):
        """Project x chunk c into dst[:, m, dst_c*HC:+HC]; bias-evac on
        `evac` engine."""
        for m in range(MT):
            ps = aux_ps.tile([128, QC], F32, tag="aux", name=f"pqk{c}m{m}")
            for ko in range(N_KO):
                nc.tensor.matmul(
                    ps[:, :HC],
                    w_s[:, ko, m * 128:(m + 1) * 128],
                    x_t[c][:, ko, :],
                    start=(ko == 0),
                    stop=(ko == N_KO - 1),
                )
            if evac is nc.scalar:
                # ACT path: out = Identity(in + bias), bias per-partition
                nc.scalar.activation(
                    dst[:, m, dst_c * HC:(dst_c + 1) * HC], ps[:, :HC],
                    mybir.ActivationFunctionType.Identity,
                    bias=b_s[:, m, None],
                )
            else:
                evac.tensor_tensor(
                    dst[:, m, dst_c * HC:(dst_c + 1) * HC], ps[:, :HC],
                    b_s[:, m, None].to_broadcast((128, HC)),
                    mybir.AluOpType.add,
                )

    v_aux = [None, None]

    def proj_v_rt(c, rt, ko_lo, ko_hi):
        """v projection for chunk c, row-tile rt, contraction slice ko_lo:hi;
        bias-evac on DVE when the group closes."""
        kt = c * 2 + rt
        if ko_lo == 0:
            v_aux[kt % 2] = aux_ps.tile([128, QC], F32, tag="aux",
                                        name=f"pv{kt}")
        ps = v_aux[kt % 2]
        for ko in range(ko_lo, ko_hi):
            nc.tensor.matmul(
                ps[:, :NV],
                xv_t[c][:, ko, rt * 128:(rt + 1) * 128],
                wv_s[:, ko],
                start=(ko == 0),
                stop=(ko == N_KO - 1),
            )
        if ko_hi == N_KO:
            nc.vector.tensor_tensor(
                v_s[:, kt, :], ps[:, :NV], bv_s[:], mybir.AluOpType.add,
            )

    # ---------------- stage A upfront ----------------
    # kT c0-7 (evac on DVE), v c0-1 (kt0-3), qT(qc0) (evac on ACT);
    # v c2-7 (kt4-15) are fillers inside the (qc0, pair0) sweep.
    qt_tiles = {}
    for c in range(2):
        proj_qk(c, xk_t, wk_s, bk_s, kT_s, c, nc.vector)
    qt_tiles[0] = qt_p.tile([128, MT, QC], F32R, tag="qt", name="qt0")
    for c in range(2):
        proj_qk(c, xq_t, wq_s, bq_s, qt_tiles[0], c % 2, nc.vector)
    for c in range(2):
        for rt in range(2):
            proj_v_rt(c, rt, 0, N_KO)
    for c in range(2, N_HC):
        proj_qk(c, xk_t, wk_s, bk_s, kT_s, c, nc.vector)

    # ---------------- attention + fillers ----------------
    # filler slots: 16 per (qc, pair) sweep.
    #  (qc0,p0): v chunks 4-7 (16 half-ko units)
    #  (qc>0,p0): out-projection of qc-1 (16 single-MM units)
    #  (*,p1) except qc3: q-projection of qc+1 (16 dual-MM units)
    osb_tiles = {}

    # v chunks 2-7 = 24 quarter-units (c, rt, half-ko); slots 0-7 take one
    # unit (chunks 2-3), slots 8-15 take two (chunks 4-7).
    def filler_vproj(s):
        units = [(2 + u // 4, (u // 2) % 2, u % 2) for u in range(24)]
        for u in ([s] if s < 8 else [8 + 2 * (s - 8), 9 + 2 * (s - 8)]):
            c, rt, half = units[u]
            proj_v_rt(c, rt, half * 4, (half + 1) * 4)

    def make_filler_oproj(qc):
        cxt = cxt_tiles[qc]

        def f(s):
            e, m = s // 2, s % 2
            if m == 0:
                osb_tiles["aux"] = aux_ps.tile([128, QC], F32, tag="aux",
                                               name=f"o{qc}e{e}")
            ps = osb_tiles["aux"]
            nc.tensor.matmul(
                ps[:],
                wo_s[:, m, e * 128:(e + 1) * 128],
                cxt[:, m, :],
                start=(m == 0),
                stop=(m == 1),
            )
            if m == 1:
                nc.vector.tensor_copy(osb_tiles[qc][:, e, :], ps[:])
                if e == D // 128 - 1:
                    nc.sync.dma_start(
                        t["oT"].rearrange("(mt p) s -> p mt s", p=128)[
                            :, :, qc * QC:(qc + 1) * QC
                        ],
                        osb_tiles[qc][:],
                    )
        return f

    def make_filler_qproj(qc):
        def f(s):
            # 32 MMs over 16 slots: (half hc, m, ko-pair); evac on Pool
            hcix, m, kp = s // 8, (s // 4) % 2, s % 4
            c = 2 * qc + hcix
            ps_key = ("q", hcix, m)
            if kp == 0:
                osb_tiles[ps_key] = aux_ps.tile([128, QC], F32, tag="aux",
                                                name=f"q{qc}h{hcix}m{m}")
            ps = osb_tiles[ps_key]
            for ko in (2 * kp, 2 * kp + 1):
                nc.tensor.matmul(
                    ps[:, :HC],
                    wq_s[:, ko, m * 128:(m + 1) * 128],
                    xq_t[c][:, ko, :],
                    start=(ko == 0),
                    stop=(ko == N_KO - 1),
                )
            if kp == 3:
                nc.vector.tensor_tensor(
                    qt_tiles[qc][:, m, hcix * HC:(hcix + 1) * HC],
                    ps[:, :HC],
                    bq_s[:, m, None].to_broadcast((128, HC)),
                    mybir.AluOpType.add,
                )
        return f

    cxt_tiles = {}
    for qc in range(N_QC):
        if qc + 1 < N_QC:
            qt_tiles[qc + 1] = qt_p.tile([128, MT, QC], F32R, tag="qt",
                                         name=f"qt{qc+1}")
        cxt_tiles[qc] = cxt_p.tile([128, MT, QC], F32R, tag="cxt",
                                   name=f"cxt{qc}")
        osb_tiles[qc] = osb_p.tile([128, D // 128, QC], F32, tag="osb",
                                   name=f"osb{qc}")
        qt = qt_tiles[qc]
        for pair in range(HEADS_PER_CORE // 2):
            # fillers: qproj(qc+1) rides the p0 sweep (a full sweep before its
            # consumers), oproj(qc-1) rides p1; qc0-p0 carries the v tail.
            if pair == 0:
                if qc == 0:
                    filler = filler_vproj
                else:
                    filler = make_filler_qproj(qc + 1) if qc + 1 < N_QC else None
            else:
                filler = make_filler_oproj(qc - 1) if qc > 0 else \
                    make_filler_qproj(1)
            m = pair
            ctx_banks = [
                ctx_ps.tile([128, QC], F32, tag=f"ctx{hl}", name=f"ctx{hl}")
                for hl in range(2)
            ]
            # PV lags LAG kt behind exp so the previous pair's ctx-bank
            # normalize chain (recip -> DRAM bcast -> mult) drains before
            # this pair's first PV needs the banks.
            LAG = 5
            pend = []

            def flush_pv(n, split=False):
                todo = pend[:max(0, len(pend) - n)]
                del pend[:len(todo)]
                hls = ([0, 1],) if not split else ([0], [1])
                for hlg in hls:
                    for ppt, pkt in todo:
                        for hl in hlg:
                            h = 2 * pair + hl
                            nc.tensor.matmul(
                                ctx_banks[hl][0:VW, :],
                                v_s[:, pkt, h * VW:(h + 1) * VW],
                                ppt[:, hl, :],
                                start=(pkt == 0),
                                stop=(pkt == N_KT - 1),
                            )
                    if split and hlg == [0]:
                        emit_norm(0)

            def emit_norm(hl):
                # recip (DVE) -> DRAM roundtrip bcast (Pool) -> mult (DVE)
                r_s = norm_p.tile([1, QC], F32, tag="r", name="r_s")
                nc.vector.reciprocal(r_s[:], ctx_banks[hl][64:65, :])
                r_d = ndram_p.tile([1, QC], F32, tag="rd", name="r_d")
                nc.gpsimd.dma_start(r_d[:], r_s[:])
                rbc = norm_p.tile([64, QC], F32, tag="rbc", name="rbc")
                nc.gpsimd.dma_start(
                    rbc[:],
                    bass.AP(
                        tensor=r_d.tensor,
                        offset=r_d.offset,
                        ap=[[0, 64]] + list(r_d.ap[1:]),
                    ),
                )
                nc.vector.tensor_tensor(
                    cxt_tiles[qc][64 * hl:64 * hl + 64, m, :],
                    ctx_banks[hl][0:64, :],
                    rbc[:],
                    mybir.AluOpType.mult,
                )

            for kt in range(N_KT):
                sc_t = sc_ps.tile([128, 2, QC], F32, tag="sc", name=f"sc{kt}")
                for hl in range(2):
                    off = 64 * hl
                    nc.tensor.matmul(
                        sc_t[:, hl, :],
                        kT_s[off:off + 64, m, kt * 128:(kt + 1) * 128],
                        qt[off:off + 64, m, :],
                        start=True,
                        stop=True,
                    )
                if filler is not None:
                    filler(kt)
                flush_pv(LAG)
                pt = pt_p.tile([128, 2, QC], F32R, tag="pt", name=f"pt{kt}")
                nc.scalar.activation(
                    pt[:], sc_t[:], mybir.ActivationFunctionType.Exp
                )
                pend.append((pt, kt))
            flush_pv(0, split=True)
            emit_norm(1)

    # tail: out-projection of qc3
    fo = make_filler_oproj(N_QC - 1)
    for s in range(16):
        fo(s)


def shard_inputs(Q, K, V, Wq, bq, Wk, bk, Wv, bv, Wo, bo):
    """Host-side shard prep. Returns per-core in_maps."""
    scale = 1.0 / np.sqrt(np.float32(DK))
    in_maps = []
    xT = {}
    for b in range(B):
        xT[b] = (
            np.ascontiguousarray(np.asarray(Q[b]).T),
            np.ascontiguousarray(np.asarray(K[b]).T),
            np.ascontiguousarray(np.asarray(V[b]).T),
        )
    for c in range(N_CORES):
        b, hg = c // HEADS_PER_CORE, c % HEADS_PER_CORE
        rows = slice(DH * hg, DH * (hg + 1))
        wqT = np.ascontiguousarray(np.asarray(Wq)[rows].T)
        wkT = np.ascontiguousarray((np.asarray(Wk)[rows] * scale).T)
        wvT = np.zeros((D, NV), np.float32)
        bv_bc = np.zeros((128, NV), np.float32)
        for i in range(HEADS_PER_CORE):
            wr = slice(DH * hg + DK * i, DH * hg + DK * (i + 1))
            wvT[:, VW * i:VW * i + DK] = np.asarray(Wv)[wr].T
            bv_bc[:, VW * i:VW * i + DK] = np.asarray(bv)[wr][None, :]
            bv_bc[:, VW * i + DK] = 1.0
        woT = np.ascontiguousarray(np.asarray(Wo)[:, rows].T)
        in_maps.append(
            {
                "xqT": xT[b][0],
                "xkT": xT[b][1],
                "xvT": xT[b][2],
                "wqT": wqT,
                "wkT": wkT,
                "wvT": wvT,
                "bq": np.ascontiguousarray(np.asarray(bq)[rows]),
                "bk": np.ascontiguousarray(np.asarray(bk)[rows] * scale),
                "bv_bc": bv_bc,
                "woT": woT,
            }
        )
    return in_maps


def unshard(results, bo):
    out = np.empty((B, S, D), np.float32)
    for b in range(B):
        acc = results[b * HEADS_PER_CORE]["oT"].astype(np.float32).copy()
        for hg in range(1, HEADS_PER_CORE):
            acc += results[b * HEADS_PER_CORE + hg]["oT"]
        out[b] = acc.T + np.asarray(bo)[None, :]
    return out


_NC_CACHE = {}


def kernel(Q, K, V, Wq, bq, Wk, bk, Wv, bv, Wo, bo):
    if "nc" not in _NC_CACHE:
        _NC_CACHE["nc"] = build_nc()
    nc = _NC_CACHE["nc"]
    in_maps = shard_inputs(Q, K, V, Wq, bq, Wk, bk, Wv, bv, Wo, bo)
    res = run_bass_kernel_spmd(nc, in_maps, core_ids=list(range(N_CORES)))
    return unshard(res.results, bo)


# revision 20
# speedup vs baseline: 1.3554x; 1.0109x over previous
"""Multi-head attention (B=2, S=2048, D=1024, H=16) on 8 Trainium2 cores.

Sharding: core c -> (batch b = c//4, head-group hg = c%4 of 4 heads, d_h=256).
Megatron-style: column-shard W_{q,k,v}, row-shard W_o; partial outputs are
summed on the host (the unshard step).

v2 restructure vs baseline: the attention inner loop is ACT(exp)-bound
(~1.04us/kt for a head-pair vs ~0.85us of PE work), so the kernel is
organized to keep ScalarE saturated with 2-bank exps ([128,1024] over both
heads of a pair) while PE fills its slack with "filler" matmuls: projections
of later chunks and the previous qc's output projection, interleaved into
the score/PV stream.  Stage-A evacuations (bias adds) run on GPSIMD (Pool),
input DMAs are spread over the ACT/DVE/Pool/SP queues in chase order.

Per-core pipeline (activations transposed "T-space", fp32r matmuls):
  kT = (Wk_hg/8) @ K_b^T        [256, 2048]
  v  = V_b @ Wv_hg^T (+ones col)[2048, 4*65]
  per 512-wide q-chunk qc, per head pair:
    per kt (128 keys): sT[2 heads] -> one 2-bank PSUM tile; exp -> pT;
    PV: ctx_h += v_h^T @ pT_h (M=65 incl denom row)
    ctx_h /= denom (DVE recip + DRAM-roundtrip partition broadcast)
  oT_partial[:, qc] = Wo_cols^T @ ctxT
Host: out[b] = (sum over the 4 cores of batch b of oT).T + bo.
"""

import numpy as np

import bass_rust
import concourse.bass as bass
import concourse.mybir as mybir
import concourse.tile as tile
from concourse.bass_utils import run_bass_kernel_spmd

F32 = mybir.dt.float32
F32R = mybir.dt.float32r
BF16 = mybir.dt.bfloat16

B, S, D = 2, 2048, 1024
H = 16
DK = 64
N_CORES = 8
HEADS_PER_CORE = 4          # d_h = 256
DH = HEADS_PER_CORE * DK    # 256
VW = DK + 1                 # v columns per head incl. ones column
NV = HEADS_PER_CORE * VW    # 260
QC = 512                    # q-chunk (PSUM bank = 512 fp32)
N_QC = S // QC              # 4
N_KT = S // 128             # 16 key tiles
N_KO = D // 128             # 8 contraction tiles for projections
MT = DH // 128              # 2 m-tiles
HC = 256                    # x half-chunk width (s columns)
N_HC = S // HC              # 8 half-chunks per input tensor


def _legalize_waits(nc):
    """walrus here allows 1 sync-wait per instruction (2 for EventSemaphore);
    Tile emits more. Spill extras onto same-engine NoOps placed just before."""
    caps = {"InstEventSemaphore": 2}
    n_nops = 0
    for f in nc.m.functions:
        for bb in f.blocks:
            insts = bb.instructions
            out = []
            changed = False
            for inst in insts:
                si = inst.sync_info
                waits = list(si.on_wait) if si is not None else []
                cap = caps.get(type(inst).__name__, 1)
                if len(waits) > cap:
                    spill, keep = waits[:-cap], waits[-cap:]
                    for w in spill:
                        nop = mybir.InstNoOp(name=f"waitfix-{n_nops}", ins=[], outs=[])
                        n_nops += 1
                        nop.engine = inst.engine
                        nop.sync_info = bass_rust.SyncInfo(on_wait=[w], on_update=[])
                        out.append(nop)
                    si.on_wait = keep
                    changed = True
                out.append(inst)
            if changed:
                insts[:] = out
    return n_nops


def build_nc(reps: int = 1):
    nc = bass.Bass(num_devices=N_CORES)

    t = {}
    t["xqT"] = nc.dram_tensor("xqT", [D, S], F32, kind="ExternalInput")
    t["xkT"] = nc.dram_tensor("xkT", [D, S], F32, kind="ExternalInput")
    t["xvT"] = nc.dram_tensor("xvT", [D, S], F32, kind="ExternalInput")
    t["wqT"] = nc.dram_tensor("wqT", [D, DH], F32, kind="ExternalInput")
    t["wkT"] = nc.dram_tensor("wkT", [D, DH], F32, kind="ExternalInput")
    t["wvT"] = nc.dram_tensor("wvT", [D, NV], F32, kind="ExternalInput")
    t["bq"] = nc.dram_tensor("bq", [DH], F32, kind="ExternalInput")
    t["bk"] = nc.dram_tensor("bk", [DH], F32, kind="ExternalInput")
    t["bv_bc"] = nc.dram_tensor("bv_bc", [128, NV], F32, kind="ExternalInput")
    t["woT"] = nc.dram_tensor("woT", [DH, D], F32, kind="ExternalInput")
    t["oT"] = nc.dram_tensor("oT", [D, S], F32, kind="ExternalOutput")

    with tile.TileContext(nc) as tc:
        _body(nc, tc, t, reps)
    _legalize_waits(nc)
    return nc


def _body(nc, tc, t, reps):
    from contextlib import ExitStack

    with ExitStack() as ctx:
        singles = ctx.enter_context(tc.tile_pool(name="singles", bufs=1))

        g = {}
        g["wq"] = singles.tile([128, N_KO, DH], F32R, name="wq_s")
        g["wk"] = singles.tile([128, N_KO, DH], F32R, name="wk_s")
        g["wv"] = singles.tile([128, N_KO, NV], F32R, name="wv_s")
        g["wo"] = singles.tile([128, MT, D], F32R, name="wo_s")
        g["bq"] = singles.tile([128, MT], F32, name="bq_s")
        g["bk"] = singles.tile([128, MT], F32, name="bk_s")
        g["bv"] = singles.tile([128, NV], F32, name="bv_s")
        # weights/biases: SP queue, ordered by first use
        nc.sync.dma_start(g["bq"][:], t["bq"].rearrange("(m p) -> p m", p=128))
        nc.sync.dma_start(g["bk"][:], t["bk"].rearrange("(m p) -> p m", p=128))
        nc.sync.dma_start(g["bv"][:], t["bv_bc"][:, :])
        nc.sync.dma_start(g["wk"][:], t["wkT"].rearrange("(ko p) m -> p ko m", p=128).bitcast(F32R))
        nc.sync.dma_start(g["wv"][:], t["wvT"].rearrange("(ko p) m -> p ko m", p=128).bitcast(F32R))

        g["kT"] = singles.tile([128, MT, S], F32R, name="kT_s")
        g["v"] = singles.tile([128, N_KT, NV], BF16, name="v_s")

        for _ in range(reps):
            with ExitStack() as ictx:
                _compute(nc, tc, ictx, t, g)


def _x_chunk(xdram, c):
    return xdram.rearrange("(ko p) s -> p ko s", p=128)[
        :, :, c * HC:(c + 1) * HC
    ].bitcast(F32R)


def _compute(nc, tc, ctx, t, g):
    wq_s, wk_s, wv_s, wo_s = g["wq"], g["wk"], g["wv"], g["wo"]
    bq_s, bk_s, bv_s = g["bq"], g["bk"], g["bv"]
    kT_s, v_s = g["kT"], g["v"]

    xk_p = ctx.enter_context(tc.tile_pool(name="xk", bufs=3))
    xv_p = ctx.enter_context(tc.tile_pool(name="xv", bufs=3))
    xq_p = ctx.enter_context(tc.tile_pool(name="xq", bufs=2))
    qt_p = ctx.enter_context(tc.tile_pool(name="qt", bufs=2))
    pt_p = ctx.enter_context(tc.tile_pool(name="pt", bufs=6))
    cxt_p = ctx.enter_context(tc.tile_pool(name="cxt", bufs=2))
    osb_p = ctx.enter_context(tc.tile_pool(name="osb", bufs=1))
    norm_p = ctx.enter_context(tc.tile_pool(name="norm", bufs=4))
    ndram_p = ctx.enter_context(tc.tile_pool(name="ndram", bufs=4, space="DRAM"))
    sc_ps = ctx.enter_context(tc.tile_pool(name="sc_ps", bufs=2, space="PSUM"))
    ctx_ps = ctx.enter_context(tc.tile_pool(name="ctx_ps", bufs=1, space="PSUM"))
    aux_ps = ctx.enter_context(tc.tile_pool(name="aux_ps", bufs=2, space="PSUM"))

    # ---------------- input DMAs (queue order = chase order) ----------------
    xk_t = [xk_p.tile([128, N_KO, HC], F32R, tag="xk", name=f"xk{c}")
            for c in range(N_HC)]
    xv_t = [xv_p.tile([128, N_KO, HC], F32R, tag="xv", name=f"xv{c}")
            for c in range(N_HC)]
    xq_t = [xq_p.tile([128, N_KO, HC], F32R, tag="xq", name=f"xq{c}")
            for c in range(N_HC)]
    for c in range(5):                       # xk c0-4 on ACT queue
        nc.scalar.dma_start(xk_t[c][:], _x_chunk(t["xkT"], c))
    for c in range(2):                       # xv c0-1 on Pool
        nc.gpsimd.dma_start(xv_t[c][:], _x_chunk(t["xvT"], c))
    nc.gpsimd.dma_start(wq_s[:], t["wqT"].rearrange("(ko p) m -> p ko m", p=128).bitcast(F32R))
    for c in range(2, 6):                    # xv c2-5 on Pool
        nc.gpsimd.dma_start(xv_t[c][:], _x_chunk(t["xvT"], c))
    # SP: xq qc0 first (attention gate), xk c5-7, then the rest by deadline
    for c in range(2):
        nc.sync.dma_start(xq_t[c][:], _x_chunk(t["xqT"], c))
    for c in range(5, N_HC):
        nc.sync.dma_start(xk_t[c][:], _x_chunk(t["xkT"], c))
    for c in range(2, 4):
        nc.sync.dma_start(xq_t[c][:], _x_chunk(t["xqT"], c))
    for c in range(2):
        nc.sync.dma_start(xv_t[6 + c][:], _x_chunk(t["xvT"], 6 + c))
    for c in range(4, 6):
        nc.sync.dma_start(xq_t[c][:], _x_chunk(t["xqT"], c))
    nc.sync.dma_start(wo_s[:], t["woT"].rearrange("(kt p) e -> p kt e", p=128).bitcast(F32R))
    for c in range(6, N_HC):
        nc.sync.dma_start(xq_t[c][:], _x_chunk(t["xqT"], c))

    # ------------- stage-A helpers -------------
    def proj_qk(c, x_t, w_s, b_s, dst, dst_c, evac):
        """Project x chunk c into dst[:, m, dst_c*HC:+HC]; bias-evac on
        `evac` engine."""
        for m in range(MT):
            ps = aux_ps.tile([128, QC], F32, tag="aux", name=f"pqk{c}m{m}")
            for ko in range(N_KO):
                nc.tensor.matmul(
                    ps[:, :HC],
                    w_s[:, ko, m * 128:(m + 1) * 128],
                    x_t[c][:, ko, :],
                    start=(ko == 0),
                    stop=(ko == N_KO - 1),
                )
            if dst is kT_s:
                evac.tensor_tensor(
                    dst[:, m, dst_c * HC:(dst_c + 1) * HC], ps[:, :HC],
                    b_s[:, m, None].to_broadcast((128, HC)),
                    mybir.AluOpType.add,
                )
            else:
                # padded qt: write each head-half into its hl slice (the
                # other 64 rows of that slice stay zero from the memset)
                for hl in range(2):
                    o = 64 * hl
                    evac.tensor_tensor(
                        dst[o:o + 64, m, hl, dst_c * HC:(dst_c + 1) * HC],
                        ps[o:o + 64, :HC],
                        b_s[o:o + 64, m, None].to_broadcast((64, HC)),
                        mybir.AluOpType.add,
                    )

    v_aux = [None, None]

    def proj_v_rt(c, rt, ko_lo, ko_hi):
        """v projection for chunk c, row-tile rt, contraction slice ko_lo:hi;
        bias-evac on DVE when the group closes."""
        kt = c * 2 + rt
        if ko_lo == 0:
            v_aux[kt % 2] = aux_ps.tile([128, QC], F32, tag="aux",
                                        name=f"pv{kt}")
        ps = v_aux[kt % 2]
        for ko in range(ko_lo, ko_hi):
            nc.tensor.matmul(
                ps[:, :NV],
                xv_t[c][:, ko, rt * 128:(rt + 1) * 128],
                wv_s[:, ko],
                start=(ko == 0),
                stop=(ko == N_KO - 1),
            )
        if ko_hi == N_KO:
            nc.vector.tensor_tensor(
                v_s[:, kt, :], ps[:, :NV], bv_s[:], mybir.AluOpType.add,
            )

    # ---------------- stage A upfront ----------------
    # kT c0-7 (evac on DVE), v c0-1 (kt0-3), qT(qc0) (evac on ACT);
    # v c2-7 (kt4-15) are fillers inside the (qc0, pair0) sweep.
    qt_tiles = {}
    for c in range(2):
        proj_qk(c, xk_t, wk_s, bk_s, kT_s, c, nc.vector)
    qt_tiles[0] = qt_p.tile([128, MT, 2, QC], F32R, tag="qt", name="qt0")
    nc.gpsimd.memset(qt_tiles[0][:].bitcast(F32), 0.0)
    for c in range(2):
        proj_qk(c, xq_t, wq_s, bq_s, qt_tiles[0], c % 2, nc.vector)
    for c in range(2):
        for rt in range(2):
            proj_v_rt(c, rt, 0, N_KO)
    for c in range(2, N_HC):
        proj_qk(c, xk_t, wk_s, bk_s, kT_s, c, nc.vector)

    # ---------------- attention + fillers ----------------
    # filler slots: 16 per (qc, pair) sweep.
    #  (qc0,p0): v chunks 4-7 (16 half-ko units)
    #  (qc>0,p0): out-projection of qc-1 (16 single-MM units)
    #  (*,p1) except qc3: q-projection of qc+1 (16 dual-MM units)
    osb_tiles = {}

    # v chunks 2-7 = 24 quarter-units (c, rt, half-ko); slots 0-7 take one
    # unit (chunks 2-3), slots 8-15 take two (chunks 4-7).
    def filler_vproj(s):
        units = [(2 + u // 4, (u // 2) % 2, u % 2) for u in range(24)]
        for u in ([s] if s < 8 else [8 + 2 * (s - 8), 9 + 2 * (s - 8)]):
            c, rt, half = units[u]
            proj_v_rt(c, rt, half * 4, (half + 1) * 4)

    def make_filler_oproj(qc):
        cxt = cxt_tiles[qc]

        def f(s):
            e, m = s // 2, s % 2
            if m == 0:
                osb_tiles["aux"] = aux_ps.tile([128, QC], F32, tag="aux",
                                               name=f"o{qc}e{e}")
            ps = osb_tiles["aux"]
            nc.tensor.matmul(
                ps[:],
                wo_s[:, m, e * 128:(e + 1) * 128],
                cxt[:, m, :],
                start=(m == 0),
                stop=(m == 1),
            )
            if m == 1:
                nc.vector.tensor_copy(osb_tiles[qc][:, e, :], ps[:])
                if e == D // 128 - 1:
                    nc.sync.dma_start(
                        t["oT"].rearrange("(mt p) s -> p mt s", p=128)[
                            :, :, qc * QC:(qc + 1) * QC
                        ],
                        osb_tiles[qc][:],
                    )
        return f

    def make_filler_qproj(qc):
        def f(s):
            # 32 MMs over 16 slots: (half hc, m, ko-pair); evac on Pool
            hcix, m, kp = s // 8, (s // 4) % 2, s % 4
            c = 2 * qc + hcix
            ps_key = ("q", hcix, m)
            if kp == 0:
                osb_tiles[ps_key] = aux_ps.tile([128, QC], F32, tag="aux",
                                                name=f"q{qc}h{hcix}m{m}")
            ps = osb_tiles[ps_key]
            for ko in (2 * kp, 2 * kp + 1):
                nc.tensor.matmul(
                    ps[:, :HC],
                    wq_s[:, ko, m * 128:(m + 1) * 128],
                    xq_t[c][:, ko, :],
                    start=(ko == 0),
                    stop=(ko == N_KO - 1),
                )
            if kp == 3:
                for hl in range(2):
                    o = 64 * hl
                    nc.vector.tensor_tensor(
                        qt_tiles[qc][o:o + 64, m, hl,
                                     hcix * HC:(hcix + 1) * HC],
                        ps[o:o + 64, :HC],
                        bq_s[o:o + 64, m, None].to_broadcast((64, HC)),
                        mybir.AluOpType.add,
                    )
        return f

    cxt_tiles = {}
    for qc in range(N_QC):
        if qc + 1 < N_QC:
            qt_tiles[qc + 1] = qt_p.tile([128, MT, 2, QC], F32R, tag="qt",
                                         name=f"qt{qc+1}")
            nc.gpsimd.memset(qt_tiles[qc + 1][:].bitcast(F32), 0.0)
        cxt_tiles[qc] = cxt_p.tile([128, MT, QC], F32R, tag="cxt",
                                   name=f"cxt{qc}")
        osb_tiles[qc] = osb_p.tile([128, D // 128, QC], F32, tag="osb",
                                   name=f"osb{qc}")
        qt = qt_tiles[qc]
        for pair in range(HEADS_PER_CORE // 2):
            # fillers: qproj(qc+1) rides the p0 sweep (a full sweep before its
            # consumers), oproj(qc-1) rides p1; qc0-p0 carries the v tail.
            if pair == 0:
                if qc == 0:
                    filler = filler_vproj
                else:
                    filler = make_filler_qproj(qc + 1) if qc + 1 < N_QC else None
            else:
                filler = make_filler_oproj(qc - 1) if qc > 0 else \
                    make_filler_qproj(1)
            m = pair
            ctx_banks = [
                ctx_ps.tile([128, QC], F32, tag=f"ctx{hl}", name=f"ctx{hl}")
                for hl in range(2)
            ]
            # PV lags LAG kt behind exp so the previous pair's ctx-bank
            # normalize chain (recip -> DRAM bcast -> mult) drains before
            # this pair's first PV needs the banks.
            LAG = 5
            pend = []

            def flush_pv(n, split=False):
                todo = pend[:max(0, len(pend) - n)]
                del pend[:len(todo)]
                hls = ([0, 1],) if not split else ([0], [1])
                for hlg in hls:
                    for ppt, pkt in todo:
                        for hl in hlg:
                            h = 2 * pair + hl
                            nc.tensor.matmul(
                                ctx_banks[hl][0:VW, :],
                                v_s[:, pkt, h * VW:(h + 1) * VW],
                                ppt[:, hl, :],
                                start=(pkt == 0),
                                stop=(pkt == N_KT - 1),
                            )
                    if split and hlg == [0]:
                        emit_norm(0)

            def emit_norm(hl):
                # recip (DVE) -> DRAM roundtrip bcast (Pool) -> mult (DVE)
                r_s = norm_p.tile([1, QC], F32, tag="r", name="r_s")
                nc.vector.reciprocal(r_s[:], ctx_banks[hl][64:65, :])
                r_d = ndram_p.tile([1, QC], F32, tag="rd", name="r_d")
                nc.gpsimd.dma_start(r_d[:], r_s[:])
                rbc = norm_p.tile([64, QC], F32, tag="rbc", name="rbc")
                nc.gpsimd.dma_start(
                    rbc[:],
                    bass.AP(
                        tensor=r_d.tensor,
                        offset=r_d.offset,
                        ap=[[0, 64]] + list(r_d.ap[1:]),
                    ),
                )
                nc.vector.tensor_tensor(
                    cxt_tiles[qc][64 * hl:64 * hl + 64, m, :],
                    ctx_banks[hl][0:64, :],
                    rbc[:],
                    mybir.AluOpType.mult,
                )

            for kt in range(N_KT):
                sc_t = sc_ps.tile([128, 2, QC], F32, tag="sc", name=f"sc{kt}")
                for hl in range(2):
                    nc.tensor.matmul(
                        sc_t[:, hl, :],
                        kT_s[:, m, kt * 128:(kt + 1) * 128],
                        qt[:, m, hl, :],
                        start=True,
                        stop=True,
                    )
                if filler is not None:
                    filler(kt)
                flush_pv(LAG)
                pt = pt_p.tile([128, 2, QC], BF16, tag="pt", name=f"pt{kt}")
                nc.scalar.activation(
                    pt[:], sc_t[:], mybir.ActivationFunctionType.Exp
                )
                pend.append((pt, kt))
            flush_pv(0, split=True)
            emit_norm(1)

    # tail: out-projection of qc3
    fo = make_filler_oproj(N_QC - 1)
    for s in range(16):
        fo(s)


def shard_inputs(Q, K, V, Wq, bq, Wk, bk, Wv, bv, Wo, bo):
    """Host-side shard prep. Returns per-core in_maps."""
    scale = 1.0 / np.sqrt(np.float32(DK))
    in_maps = []
    xT = {}
    for b in range(B):
        xT[b] = (
            np.ascontiguousarray(np.asarray(Q[b]).T),
            np.ascontiguousarray(np.asarray(K[b]).T),
            np.ascontiguousarray(np.asarray(V[b]).T),
        )
    for c in range(N_CORES):
        b, hg = c // HEADS_PER_CORE, c % HEADS_PER_CORE
        rows = slice(DH * hg, DH * (hg + 1))
        wqT = np.ascontiguousarray(np.asarray(Wq)[rows].T)
        wkT = np.ascontiguousarray((np.asarray(Wk)[rows] * scale).T)
        wvT = np.zeros((D, NV), np.float32)
        bv_bc = np.zeros((128, NV), np.float32)
        for i in range(HEADS_PER_CORE):
            wr = slice(DH * hg + DK * i, DH * hg + DK * (i + 1))
            wvT[:, VW * i:VW * i + DK] = np.asarray(Wv)[wr].T
            bv_bc[:, VW * i:VW * i + DK] = np.asarray(bv)[wr][None, :]
            bv_bc[:, VW * i + DK] = 1.0
        woT = np.ascontiguousarray(np.asarray(Wo)[:, rows].T)
        in_maps.append(
            {
                "xqT": xT[b][0],
                "xkT": xT[b][1],
                "xvT": xT[b][2],
                "wqT": wqT,
                "wkT": wkT,
                "wvT": wvT,
                "bq": np.ascontiguousarray(np.asarray(bq)[rows]),
                "bk": np.ascontiguousarray(np.asarray(bk)[rows] * scale),
                "bv_bc": bv_bc,
                "woT": woT,
            }
        )
    return in_maps


def unshard(results, bo):
    out = np.empty((B, S, D), np.float32)
    for b in range(B):
        acc = results[b * HEADS_PER_CORE]["oT"].astype(np.float32).copy()
        for hg in range(1, HEADS_PER_CORE):
            acc += results[b * HEADS_PER_CORE + hg]["oT"]
        out[b] = acc.T + np.asarray(bo)[None, :]
    return out


_NC_CACHE = {}


def kernel(Q, K, V, Wq, bq, Wk, bk, Wv, bv, Wo, bo):
    if "nc" not in _NC_CACHE:
        _NC_CACHE["nc"] = build_nc()
    nc = _NC_CACHE["nc"]
    in_maps = shard_inputs(Q, K, V, Wq, bq, Wk, bk, Wv, bv, Wo, bo)
    res = run_bass_kernel_spmd(nc, in_maps, core_ids=list(range(N_CORES)))
    return unshard(res.results, bo)
